# revision 2
# baseline (speedup 1.0000x reference)
"""Trainium2 Bass kernel for nn_EstraNet_1443109012284.

Mathematical reduction: the reference's FAVOR+/trig branch (phi_q, aux_q/k,
fr_q/k, aux_A, A) does not feed the output.  The output is exactly

    out[b,n,d] = sum_{h,c} W_o[h,c,d] * norma[h] * sum_{d'} W_v[d',h,c] * x[b,n,d']
               = (x @ M)[b,n,d],   M[d',d] = sum_{h,c} W_v[d',h,c] norma[h] W_o[h,c,d]

with norma[h] = || sum_d s_p[h] W_p[d,h,:] beta_p[d] ||_2.

M is a tiny [512,512] matrix folded on the host; the device does the single
big GEMM  y[32768,512] = x[32768,512] @ M[512,512]  data-parallel over rows:
each of the 8 cores handles 4096 rows.

Device design (per core), v2:
- yT[d', n] = sum_k M[k, d'] * xT[k, n]; lhsT (stationary) = M chunk
  [128k x 128d'], rhs (moving) = xT slices [128k x 512n] fed pre-transposed
  from the host.
- x arrives as 4 k-chunks [128, 4096] (1 MB each; k=0 split in halves for an
  earlier first MM), spread over the sync/scalar HWDGE queues + one SWDGE.
- Loop: d-block outer, k middle, j (n-chunk) inner; 8 PSUM banks hold one
  d-block's 8 n-chunks while k accumulates.
- PSUM drained by DVE only (single PSUM reader; ~267 ns per [128,512]
  fp32->fp16 copy chases the MM stream with no bank-reuse stall).
- Output: one 1 MB store per d-block for d<3; the last d-block stores two
  512 KB halves on the two HWDGE queues so the tail after the last MM is
  copy+store of 512 KB, not 1 MB.
- PE warmed up with dummy matmuls (dep: a memset tile only) during the
  input-DMA window so the HAM clock ramp doesn't tax real work.
- fp16: x, M, y all fp16, M pre-scaled by an exact power of two so M / y
  avoid the fp16 subnormal range; host multiplies the scale back out.
"""

import os as _os
import sys

sys.path.insert(0, "/opt/trn_rl_repo")

import numpy as np

import concourse.bass as bass
import concourse.tile as tile
from concourse import bacc, mybir
from concourse.bass_utils import run_bass_kernel_spmd

N_CORES = 8
ROWS = 32768           # B*N = 8*4096
RPC = ROWS // N_CORES  # rows per core = 4096
D = 512
KC = 4                 # contraction chunks of 128
DT = D // 128          # output row-blocks = 4
NJ = RPC // 512        # moving chunks of 512 per d-block = 8

COMPUTE_DTYPE = _os.environ.get("KERNEL_DTYPE", "fp16")
N_WARM = int(_os.environ.get("KERNEL_NWARM", "6"))

_DT = {
    "fp32": mybir.dt.float32,
    "f32r": mybir.dt.float32r,
    "bf16": mybir.dt.bfloat16,
    "fp16": mybir.dt.float16,
}


def _np_dtype(token):
    if token == "bf16":
        import ml_dtypes

        return ml_dtypes.bfloat16
    if token == "fp16":
        return np.float16
    return np.float32


def _build(token):
    dt_in = _DT[token]
    dt_out = mybir.dt.float16 if token == "fp16" else mybir.dt.float32
    nc = bacc.Bacc("TRN2", target_bir_lowering=False)
    # x pre-transposed, k-chunk major: [KC, 128, RPC]; each k-chunk is one
    # contiguous 1 MB block (8 KB per partition line)
    xt = nc.dram_tensor("xt", [KC, 128, RPC], dt_in, kind="ExternalInput")
    mm = nc.dram_tensor("mm", [128, KC, D], dt_in, kind="ExternalInput")
    yt = nc.dram_tensor("yt", [D, RPC], dt_out, kind="ExternalOutput")

    with tile.TileContext(nc) as tc:
        with (
            tc.tile_pool(name="xp", bufs=1) as xp,
            tc.tile_pool(name="mp", bufs=1) as mp,
            tc.tile_pool(name="op", bufs=4) as op,
            tc.tile_pool(name="pp", bufs=8, space="PSUM") as pp,
        ):
            # PE warmup: matmuls that depend only on a memset tile start
            # right after engine code load and burn the HAM cold-clock ramp
            # while the x DMAs are still in flight.
            wz = mp.tile([128, 512], mybir.dt.bfloat16, name="wz")
            nc.gpsimd.memset(wz[:], 1.0)
            warm = pp.tile([128, 512], mybir.dt.float32, tag="ps", name="warm")
            for w in range(N_WARM):
                nc.tensor.matmul(
                    warm[:], wz[:, 0:128], wz[:], start=True, stop=True
                )

            # input DMAs: m + k=0 halves land first (first MMs), k=1..3
            # spread over scalar / gpsimd / sync so they land k-ordered
            m_sb = mp.tile([128, KC, D], dt_in, name="m_sb")
            nc.scalar.dma_start(out=m_sb[:], in_=mm[:])

            HALF = RPC // 2
            x0a = xp.tile([128, HALF], dt_in, name="x0a")
            nc.sync.dma_start(out=x0a[:], in_=xt[0, :, 0:HALF])
            x0b = xp.tile([128, HALF], dt_in, name="x0b")
            nc.sync.dma_start(out=x0b[:], in_=xt[0, :, HALF:RPC])
            x1 = xp.tile([128, RPC], dt_in, name="x1")
            nc.scalar.dma_start(out=x1[:], in_=xt[1])
            x3 = xp.tile([128, RPC], dt_in, name="x3")
            nc.gpsimd.dma_start(out=x3[:], in_=xt[3])
            x2 = xp.tile([128, RPC], dt_in, name="x2")
            nc.sync.dma_start(out=x2[:], in_=xt[2])

            def rhs(k, j):
                c0 = j * 512
                if k == 0:
                    if c0 < HALF:
                        return x0a[:, c0 : c0 + 512]
                    return x0b[:, c0 - HALF : c0 - HALF + 512]
                return {1: x1, 2: x2, 3: x3}[k][:, c0 : c0 + 512]

            for d in range(DT):
                d0 = d * 128
                last = d == DT - 1
                pss = [
                    pp.tile([128, 512], mybir.dt.float32, tag="ps", name=f"ps_{d}_{j}")
                    for j in range(NJ)
                ]
                ot = op.tile([128, RPC], dt_out, name=f"ot{d}", tag="ot")
                for k in range(KC):
                    for j in range(NJ):
                        nc.tensor.matmul(
                            pss[j][:],
                            m_sb[:, k, d0 : d0 + 128],
                            rhs(k, j),
                            start=(k == 0),
                            stop=(k == KC - 1),
                        )
                # single PSUM reader: DVE copies chase the k=3 MMs
                for j in range(NJ):
                    nc.vector.tensor_copy(
                        ot[:, j * 512 : (j + 1) * 512], pss[j][:]
                    )
                if last:
                    # two 512 KB halves on the two (now idle) HWDGE queues:
                    # first half streams while the second half's copies run
                    nc.scalar.dma_start(
                        out=yt[d0 : d0 + 128, 0:HALF], in_=ot[:, 0:HALF]
                    )
                    nc.sync.dma_start(
                        out=yt[d0 : d0 + 128, HALF:RPC], in_=ot[:, HALF:RPC]
                    )
                else:
                    eng = {0: nc.sync, 1: nc.gpsimd, 2: nc.scalar}[d]
                    eng.dma_start(out=yt[d0 : d0 + 128, :], in_=ot[:])
    nc.compile()
    return nc


def _fold_m(W_v, s_p, W_p, beta_p, W_o):
    """Host-side constant folding of the tiny parameter tensors into M."""
    W_v = np.asarray(W_v, dtype=np.float64)
    s_p = np.asarray(s_p, dtype=np.float64)
    W_p = np.asarray(W_p, dtype=np.float64)
    beta_p = np.asarray(beta_p, dtype=np.float64)
    W_o = np.asarray(W_o, dtype=np.float64)
    phi = np.einsum("h,dhc,d->hc", s_p, W_p, beta_p)
    norma = np.linalg.norm(phi, axis=1)  # [h]
    M = np.einsum("dhc,h,hce->de", W_v, norma, W_o)  # [512, 512]
    return M.astype(np.float32)


_prog_cache = {}
_last_in_maps = None  # kept for test.py profiling reuse
_last_result = None


def _run(in_maps, token, **kwargs):
    if token not in _prog_cache:
        _prog_cache[token] = _build(token)
    return run_bass_kernel_spmd(_prog_cache[token], in_maps, list(range(N_CORES)), **kwargs)


def kernel(x, W_v, s_p, c_p, W_p, W_A, W_o, beta_p, beta_i_p, **_unused):
    global _last_in_maps, _last_result
    token = COMPUTE_DTYPE
    np_dt = _np_dtype(token)

    x = np.asarray(x, dtype=np.float32)
    M = _fold_m(W_v, s_p, W_p, beta_p, W_o)

    # fp16 path: scale M by an exact power of two so M entries and y values
    # sit in fp16 normal range; undo on the host after the run
    out_unscale = 1.0
    if token == "fp16":
        amax = float(np.abs(M).max())
        if amax > 0:
            e = int(np.floor(-np.log2(amax)))
            M = M * np.float32(2.0**e)
            out_unscale = 2.0**-e

    B, N, Dd = x.shape
    assert B * N == ROWS and Dd == D, (x.shape,)

    mmc = np.ascontiguousarray(M.reshape(KC, 128, D).transpose(1, 0, 2)).astype(np_dt)
    xf = x.reshape(ROWS, D)

    in_maps = []
    for c in range(N_CORES):
        sh = xf[c * RPC : (c + 1) * RPC]               # [4096, 512]
        xT = sh.T.astype(np_dt)                        # [512, 4096]
        xs = np.ascontiguousarray(xT.reshape(KC, 128, RPC))
        in_maps.append({"xt": xs, "mm": mmc})

    _last_in_maps = in_maps
    res = _run(in_maps, token)
    _last_result = res
    out = np.empty((ROWS, D), dtype=np.float32)
    for c in range(N_CORES):
        yc = res.results[c]["yt"].astype(np.float32)
        if out_unscale != 1.0:
            yc *= np.float32(out_unscale)
        out[c * RPC : (c + 1) * RPC] = yc.T
    return out.reshape(B, N, D)


if __name__ == "__main__":
    # smoke test with random data
    rng = np.random.default_rng(0)
    x = rng.standard_normal((8, 4096, 512)).astype(np.float32)
    W_v = rng.standard_normal((512, 8, 64)).astype(np.float32) * 0.01
    s_p = np.ones((8,), np.float32)
    c_p = np.ones((8,), np.float32)
    W_p = rng.standard_normal((512, 8, 64)).astype(np.float32) * 0.01
    W_A = rng.standard_normal((256, 64)).astype(np.float32)
    W_o = rng.standard_normal((8, 64, 512)).astype(np.float32) * 0.01
    beta_p = rng.standard_normal((512,)).astype(np.float32) * 1e-5
    beta_i_p = rng.standard_normal((4096, 512)).astype(np.float32) * 1e-5
    out = kernel(x, W_v=W_v, s_p=s_p, c_p=c_p, W_p=W_p, W_A=W_A, W_o=W_o,
                 beta_p=beta_p, beta_i_p=beta_i_p)
    M = _fold_m(W_v, s_p, W_p, beta_p, W_o)
    exp = (x.reshape(-1, 512).astype(np.float64) @ M.astype(np.float64)).reshape(8, 4096, 512)
    err = np.abs(out - exp).max() / (np.abs(exp).max() + 1e-30)
    print("smoke rel err:", err)


# revision 3
# speedup vs baseline: 1.1529x; 1.1529x over previous
"""Trainium2 Bass kernel for nn_EstraNet_1443109012284.

Mathematical reduction: the reference's FAVOR+/trig branch (phi_q, aux_q/k,
fr_q/k, aux_A, A) does not feed the output.  The output is exactly

    out[b,n,d] = sum_{h,c} W_o[h,c,d] * norma[h] * sum_{d'} W_v[d',h,c] * x[b,n,d']
               = (x @ M)[b,n,d],   M[d',d] = sum_{h,c} W_v[d',h,c] norma[h] W_o[h,c,d]

with norma[h] = || sum_d s_p[h] W_p[d,h,:] beta_p[d] ||_2.

M is a tiny [512,512] matrix folded on the host; the device does the single
big GEMM  y[32768,512] = x[32768,512] @ M[512,512]  data-parallel over rows:
each of the 8 cores handles 4096 rows.

Device design (per core): compute yT[d, n] = sum_k M[k, d] * xT[k, n]
- lhsT (stationary) = M chunk [128k x 128d]; rhs (moving) = xT quarter
  [128k x 512n], fed pre-transposed from the host (no on-device transpose).
- Phase structure (h-quarter outer, d-block inner) consumes x in DMA
  arrival order: the first phase only needs the first quarter-wave.
- First wave (h=0) arrives as 8 half-quarters of 128 KB and m as 4 per-k
  chunks, interleaved so the k-major MM order never waits; later waves are
  full 256 KB quarters.
- PSUM->SBUF copies all on ONE engine (ACT): PE drain + a single reader
  share PSUM fine; two concurrent readers throttle the PE ~2.3x. (DVE CAST
  measured 681 ns per [128,512] vs ACT's 474 ns - ACT is the fast reader.)
- Last phase: half-granularity copies, with the two 128 KB stores issued on
  the two idle HWDGE queues (scalar + sync) in parallel so the kernel tail
  after the last matmul is one copy + one small store.
- PE warmed up with dummy matmuls (dep: a memset tile only) during the
  input-DMA window so the HAM clock ramp doesn't tax real work.
- fp16 path (default): x, M, y all fp16, M pre-scaled by an exact power of
  two so M / y avoid the fp16 subnormal range; host multiplies the scale
  back out.  fp16 keeps 10 mantissa bits (vs bf16's 7) and halves output
  DMA vs fp32 -> kernel is PE-bound at ~216ns per [128x128]x[128x512] MM.
"""

import os as _os
import sys

sys.path.insert(0, "/opt/trn_rl_repo")

import numpy as np

import concourse.bass as bass
import concourse.tile as tile
from concourse import bacc, mybir
from concourse.bass_utils import run_bass_kernel_spmd

N_CORES = 8
ROWS = 32768           # B*N = 8*4096
RPC = ROWS // N_CORES  # rows per core = 4096
D = 512
KC = 4                 # contraction chunks of 128
DT = D // 128          # output row-blocks = 4
HB = 4                 # n-quarters per stripe
HW = RPC // HB         # 1024 columns per quarter
JH = HW // 512         # moving chunks of 512 per phase = 2

COMPUTE_DTYPE = _os.environ.get("KERNEL_DTYPE", "fp16")
N_WARM = int(_os.environ.get("KERNEL_NWARM", "8"))

_DT = {
    "fp32": mybir.dt.float32,
    "f32r": mybir.dt.float32r,
    "bf16": mybir.dt.bfloat16,
    "fp16": mybir.dt.float16,
}


def _np_dtype(token):
    if token == "bf16":
        import ml_dtypes

        return ml_dtypes.bfloat16
    if token == "fp16":
        return np.float16
    return np.float32


def _build(token):
    dt_in = _DT[token]
    dt_out = mybir.dt.float16 if token == "fp16" else mybir.dt.float32
    nc = bacc.Bacc("TRN2", target_bir_lowering=False)
    # x pre-transposed, [k-chunk, quarter, 128, 1024]: each quarter-stripe is
    # one contiguous DMA
    xt = nc.dram_tensor("xt", [KC, HB, 128, HW], dt_in, kind="ExternalInput")
    mm = nc.dram_tensor("mm", [128, KC, D], dt_in, kind="ExternalInput")
    yt = nc.dram_tensor("yt", [D, RPC], dt_out, kind="ExternalOutput")

    with tile.TileContext(nc) as tc:
        with (
            tc.tile_pool(name="xp", bufs=1) as xp,
            tc.tile_pool(name="mp", bufs=1) as mp,
            tc.tile_pool(name="op", bufs=4) as op,
            tc.tile_pool(name="pp", bufs=8, space="PSUM") as pp,
        ):
            # PE warmup: matmuls that depend only on a memset tile start at
            # ~7.5us (right after engine code load) and burn the HAM
            # cold-clock ramp while the x DMAs are still in flight.
            wz = mp.tile([128, 512], mybir.dt.bfloat16, name="wz")
            nc.gpsimd.memset(wz[:], 1.0)
            warm = pp.tile([128, 512], mybir.dt.float32, tag="ps", name="warm")
            for w in range(N_WARM):
                nc.tensor.matmul(
                    warm[:], wz[:, 0:128], wz[:], start=True, stop=True
                )

            # First wave: m split per-k and the h=0 quarters split in halves,
            # interleaved across the two HWDGE queues so the k-major MM order
            # of the first phases never waits on a late chunk.
            m_sb = mp.tile([128, KC, D], dt_in, name="m_sb")
            x_sb = {}
            for k in range(KC):
                t = xp.tile([128, HW], dt_in, tag=f"x{k}0", name=f"x{k}0")
                x_sb[(k, 0)] = t
            HH = HW // 2
            # sync queue: m0, k0a, k0b, k2a, k2b ; scalar: m1, k1a, m2, k1b, m3, k3a, k3b
            nc.sync.dma_start(out=m_sb[:, 0], in_=mm[:, 0])
            nc.scalar.dma_start(out=m_sb[:, 1], in_=mm[:, 1])
            nc.sync.dma_start(out=x_sb[(0, 0)][:, 0:HH], in_=xt[0, 0, :, 0:HH])
            nc.scalar.dma_start(out=x_sb[(1, 0)][:, 0:HH], in_=xt[1, 0, :, 0:HH])
            nc.sync.dma_start(out=m_sb[:, 2], in_=mm[:, 2])
            nc.scalar.dma_start(out=m_sb[:, 3], in_=mm[:, 3])
            nc.sync.dma_start(out=x_sb[(2, 0)][:, 0:HH], in_=xt[2, 0, :, 0:HH])
            nc.scalar.dma_start(out=x_sb[(3, 0)][:, 0:HH], in_=xt[3, 0, :, 0:HH])
            nc.sync.dma_start(out=x_sb[(0, 0)][:, HH:HW], in_=xt[0, 0, :, HH:HW])
            nc.scalar.dma_start(out=x_sb[(1, 0)][:, HH:HW], in_=xt[1, 0, :, HH:HW])
            nc.sync.dma_start(out=x_sb[(2, 0)][:, HH:HW], in_=xt[2, 0, :, HH:HW])
            nc.scalar.dma_start(out=x_sb[(3, 0)][:, HH:HW], in_=xt[3, 0, :, HH:HW])

            # Later waves: full 256 KB quarters, h-major so arrival matches
            # phase consumption order.
            for h in range(1, HB):
                for k in range(KC):
                    t = xp.tile([128, HW], dt_in, tag=f"x{k}{h}", name=f"x{k}{h}")
                    eng = nc.sync if (h * KC + k) % 2 == 0 else nc.scalar
                    eng.dma_start(out=t[:], in_=xt[k, h])
                    x_sb[(k, h)] = t

            # phases: h outer (first phase only needs the first wave),
            # d inner.  k-major MM order (4 weight switches per phase, banks
            # finish staggered); last phase: copies + parallel half stores.
            NPH = HB * DT
            for ph in range(NPH):
                h, d = divmod(ph, DT)
                d0 = d * 128
                last = ph == NPH - 1
                ot = op.tile([128, HW], dt_out, name=f"ot{ph}", tag="ot")
                pss = [
                    pp.tile([128, 512], mybir.dt.float32, tag="ps", name=f"ps_{h}_{d}_{j}")
                    for j in range(JH)
                ]
                # alternate output DMAs between the sync HWDGE queue and the
                # gpsimd SWDGE rings (POOL sequencer is otherwise idle) so
                # input and output streams don't serialize on one ring
                oeng = nc.gpsimd if ph % 2 == 0 else nc.sync
                for k in range(KC):
                    for j in range(JH):
                        nc.tensor.matmul(
                            pss[j][:],
                            m_sb[:, k, d0 : d0 + 128],
                            x_sb[(k, h)][:, j * 512 : (j + 1) * 512],
                            start=(k == 0),
                            stop=(k == KC - 1),
                        )
                if last:
                    # final phase: copy each 512-col bank on ACT, store each
                    # half on its own (idle, warm) HWDGE queue in parallel so
                    # the tail is one copy + one 128 KB store
                    nc.scalar.copy(ot[:, 0:512], pss[0][:])
                    nc.scalar.dma_start(
                        out=yt[d0 : d0 + 128, h * HW : h * HW + 512],
                        in_=ot[:, 0:512],
                    )
                    nc.scalar.copy(ot[:, 512:1024], pss[1][:])
                    nc.sync.dma_start(
                        out=yt[d0 : d0 + 128, h * HW + 512 : (h + 1) * HW],
                        in_=ot[:, 512:1024],
                    )
                else:
                    for j in range(JH):
                        nc.scalar.copy(ot[:, j * 512 : (j + 1) * 512], pss[j][:])
                    oeng.dma_start(
                        out=yt[d0 : d0 + 128, h * HW : (h + 1) * HW], in_=ot[:]
                    )
    nc.compile()
    return nc


def _fold_m(W_v, s_p, W_p, beta_p, W_o):
    """Host-side constant folding of the tiny parameter tensors into M."""
    W_v = np.asarray(W_v, dtype=np.float64)
    s_p = np.asarray(s_p, dtype=np.float64)
    W_p = np.asarray(W_p, dtype=np.float64)
    beta_p = np.asarray(beta_p, dtype=np.float64)
    W_o = np.asarray(W_o, dtype=np.float64)
    phi = np.einsum("h,dhc,d->hc", s_p, W_p, beta_p)
    norma = np.linalg.norm(phi, axis=1)  # [h]
    M = np.einsum("dhc,h,hce->de", W_v, norma, W_o)  # [512, 512]
    return M.astype(np.float32)


_prog_cache = {}
_last_in_maps = None  # kept for test.py profiling reuse
_last_result = None


def _run(in_maps, token, **kwargs):
    if token not in _prog_cache:
        _prog_cache[token] = _build(token)
    return run_bass_kernel_spmd(_prog_cache[token], in_maps, list(range(N_CORES)), **kwargs)


def kernel(x, W_v, s_p, c_p, W_p, W_A, W_o, beta_p, beta_i_p, **_unused):
    global _last_in_maps, _last_result
    token = COMPUTE_DTYPE
    np_dt = _np_dtype(token)

    x = np.asarray(x, dtype=np.float32)
    M = _fold_m(W_v, s_p, W_p, beta_p, W_o)

    # fp16 path: scale M by an exact power of two so M entries and y values
    # sit in fp16 normal range; undo on the host after the run
    out_unscale = 1.0
    if token == "fp16":
        amax = float(np.abs(M).max())
        if amax > 0:
            e = int(np.floor(-np.log2(amax)))
            M = M * np.float32(2.0**e)
            out_unscale = 2.0**-e

    B, N, Dd = x.shape
    assert B * N == ROWS and Dd == D, (x.shape,)

    mmc = np.ascontiguousarray(M.reshape(KC, 128, D).transpose(1, 0, 2)).astype(np_dt)
    xf = x.reshape(ROWS, D)

    in_maps = []
    for c in range(N_CORES):
        sh = xf[c * RPC : (c + 1) * RPC]               # [4096, 512]
        xT = sh.T.astype(np_dt)                        # [512, 4096]
        # [KC, 128, HB, HW] -> [KC, HB, 128, HW], each quarter contiguous
        xs = np.ascontiguousarray(
            xT.reshape(KC, 128, HB, HW).transpose(0, 2, 1, 3)
        )
        in_maps.append({"xt": xs, "mm": mmc})

    _last_in_maps = in_maps
    res = _run(in_maps, token)
    _last_result = res
    out = np.empty((ROWS, D), dtype=np.float32)
    for c in range(N_CORES):
        yc = res.results[c]["yt"].astype(np.float32)
        if out_unscale != 1.0:
            yc *= np.float32(out_unscale)
        out[c * RPC : (c + 1) * RPC] = yc.T
    return out.reshape(B, N, D)


if __name__ == "__main__":
    # smoke test with random data
    rng = np.random.default_rng(0)
    x = rng.standard_normal((8, 4096, 512)).astype(np.float32)
    W_v = rng.standard_normal((512, 8, 64)).astype(np.float32) * 0.01
    s_p = np.ones((8,), np.float32)
    c_p = np.ones((8,), np.float32)
    W_p = rng.standard_normal((512, 8, 64)).astype(np.float32) * 0.01
    W_A = rng.standard_normal((256, 64)).astype(np.float32)
    W_o = rng.standard_normal((8, 64, 512)).astype(np.float32) * 0.01
    beta_p = rng.standard_normal((512,)).astype(np.float32) * 1e-5
    beta_i_p = rng.standard_normal((4096, 512)).astype(np.float32) * 1e-5
    out = kernel(x, W_v=W_v, s_p=s_p, c_p=c_p, W_p=W_p, W_A=W_A, W_o=W_o,
                 beta_p=beta_p, beta_i_p=beta_i_p)
    M = _fold_m(W_v, s_p, W_p, beta_p, W_o)
    exp = (x.reshape(-1, 512).astype(np.float64) @ M.astype(np.float64)).reshape(8, 4096, 512)
    err = np.abs(out - exp).max() / (np.abs(exp).max() + 1e-30)
    print("smoke rel err:", err)


# revision 4
# speedup vs baseline: 1.1902x; 1.0324x over previous
"""Trainium2 Bass kernel for nn_EstraNet_1443109012284.

Mathematical reduction: the reference's FAVOR+/trig branch (phi_q, aux_q/k,
fr_q/k, aux_A, A) does not feed the output.  The output is exactly

    out[b,n,d] = sum_{h,c} W_o[h,c,d] * norma[h] * sum_{d'} W_v[d',h,c] * x[b,n,d']
               = (x @ M)[b,n,d],   M[d',d] = sum_{h,c} W_v[d',h,c] norma[h] W_o[h,c,d]

with norma[h] = || sum_d s_p[h] W_p[d,h,:] beta_p[d] ||_2.

M is a tiny [512,512] matrix folded on the host; the device does the single
big GEMM  y[32768,512] = x[32768,512] @ M[512,512]  data-parallel over rows:
each of the 8 cores handles 4096 rows.

Device design (per core): compute yT[d, n] = sum_k M[k, d] * xT[k, n]
- lhsT (stationary) = M chunk [128k x 128d]; rhs (moving) = xT quarter
  [128k x 512n], fed pre-transposed from the host (no on-device transpose).
- Phase structure (h-quarter outer, d-block inner) consumes x in DMA
  arrival order: the first phase only needs the first quarter-wave.
- First wave (h=0) arrives as 8 half-quarters of 128 KB and m as 4 per-k
  chunks, interleaved so the k-major MM order never waits; later waves are
  full 256 KB quarters.
- PSUM->SBUF copies all on ONE engine (ACT): PE drain + a single reader
  share PSUM fine; two concurrent readers throttle the PE ~2.3x. (DVE CAST
  measured 681 ns per [128,512] vs ACT's 474 ns - ACT is the fast reader.)
- Last phase: half-granularity copies, with the two 128 KB stores issued on
  the two idle HWDGE queues (scalar + sync) in parallel so the kernel tail
  after the last matmul is one copy + one small store.
- PE warmed up with dummy matmuls (dep: a memset tile only) during the
  input-DMA window so the HAM clock ramp doesn't tax real work.
- fp16 path (default): x, M, y all fp16, M pre-scaled by an exact power of
  two so M / y avoid the fp16 subnormal range; host multiplies the scale
  back out.  fp16 keeps 10 mantissa bits (vs bf16's 7) and halves output
  DMA vs fp32 -> kernel is PE-bound at ~216ns per [128x128]x[128x512] MM.
"""

import os as _os
import sys

sys.path.insert(0, "/opt/trn_rl_repo")

import numpy as np

import concourse.bass as bass
import concourse.tile as tile
from concourse import bacc, mybir
from concourse.bass_utils import run_bass_kernel_spmd

N_CORES = 8
ROWS = 32768           # B*N = 8*4096
RPC = ROWS // N_CORES  # rows per core = 4096
D = 512
KC = 4                 # contraction chunks of 128
DT = D // 128          # output row-blocks = 4
HB = 4                 # n-quarters per stripe
HW = RPC // HB         # 1024 columns per quarter
JH = HW // 512         # moving chunks of 512 per phase = 2

COMPUTE_DTYPE = _os.environ.get("KERNEL_DTYPE", "fp16")
N_WARM = int(_os.environ.get("KERNEL_NWARM", "8"))

_DT = {
    "fp32": mybir.dt.float32,
    "f32r": mybir.dt.float32r,
    "bf16": mybir.dt.bfloat16,
    "fp16": mybir.dt.float16,
}


def _np_dtype(token):
    if token == "bf16":
        import ml_dtypes

        return ml_dtypes.bfloat16
    if token == "fp16":
        return np.float16
    return np.float32


def _build(token):
    dt_in = _DT[token]
    dt_out = mybir.dt.float16 if token == "fp16" else mybir.dt.float32
    nc = bacc.Bacc("TRN2", target_bir_lowering=False)
    # x pre-transposed, [k-chunk, quarter, 128, 1024]: each quarter-stripe is
    # one contiguous DMA
    xt = nc.dram_tensor("xt", [KC, HB, 128, HW], dt_in, kind="ExternalInput")
    mm = nc.dram_tensor("mm", [128, KC, D], dt_in, kind="ExternalInput")
    yt = nc.dram_tensor("yt", [D, RPC], dt_out, kind="ExternalOutput")

    with tile.TileContext(nc) as tc:
        with (
            tc.tile_pool(name="xp", bufs=1) as xp,
            tc.tile_pool(name="mp", bufs=1) as mp,
            tc.tile_pool(name="op", bufs=4) as op,
            tc.tile_pool(name="pp", bufs=8, space="PSUM") as pp,
        ):
            # PE warmup: matmuls that depend only on a memset tile start at
            # ~7.5us (right after engine code load) and burn the HAM
            # cold-clock ramp while the x DMAs are still in flight.
            wz = mp.tile([128, 512], mybir.dt.bfloat16, name="wz")
            nc.gpsimd.memset(wz[:], 1.0)
            warm = pp.tile([128, 512], mybir.dt.float32, tag="ps", name="warm")
            for w in range(N_WARM):
                nc.tensor.matmul(
                    warm[:], wz[:, 0:128], wz[:], start=True, stop=True
                )

            # First wave: m split per-k (4 x 128 KB on sync) and the h=0
            # quarters k-ordered on scalar, so the k-major MM order of the
            # first phases sees m0+k0 earliest and never waits on a late
            # chunk.  k3h0 rides sync (scalar's 3-deep chain would land it
            # late).
            m_sb = mp.tile([128, KC, D], dt_in, name="m_sb")
            x_sb = {}
            for k in range(KC):
                t = xp.tile([128, HW], dt_in, tag=f"x{k}0", name=f"x{k}0")
                x_sb[(k, 0)] = t
            nc.sync.dma_start(out=m_sb[:, 0], in_=mm[:, 0])
            nc.scalar.dma_start(out=x_sb[(0, 0)][:], in_=xt[0, 0])
            nc.sync.dma_start(out=m_sb[:, 1], in_=mm[:, 1])
            nc.scalar.dma_start(out=x_sb[(1, 0)][:], in_=xt[1, 0])
            nc.sync.dma_start(out=x_sb[(3, 0)][:], in_=xt[3, 0])
            nc.scalar.dma_start(out=x_sb[(2, 0)][:], in_=xt[2, 0])
            nc.sync.dma_start(out=m_sb[:, 2], in_=mm[:, 2])
            nc.sync.dma_start(out=m_sb[:, 3], in_=mm[:, 3])

            # Later waves: full 256 KB quarters, h-major so arrival matches
            # phase consumption order.
            for h in range(1, HB):
                for k in range(KC):
                    t = xp.tile([128, HW], dt_in, tag=f"x{k}{h}", name=f"x{k}{h}")
                    eng = nc.sync if (h * KC + k) % 2 == 0 else nc.scalar
                    eng.dma_start(out=t[:], in_=xt[k, h])
                    x_sb[(k, h)] = t

            # phases: h outer (first phase only needs the first wave),
            # d inner.  k-major MM order (4 weight switches per phase, banks
            # finish staggered); last phase: copies + parallel half stores.
            NPH = HB * DT
            for ph in range(NPH):
                h, d = divmod(ph, DT)
                d0 = d * 128
                last = ph == NPH - 1
                ot = op.tile([128, HW], dt_out, name=f"ot{ph}", tag="ot")
                pss = [
                    pp.tile([128, 512], mybir.dt.float32, tag="ps", name=f"ps_{h}_{d}_{j}")
                    for j in range(JH)
                ]
                # alternate output DMAs between the sync HWDGE queue and the
                # gpsimd SWDGE rings (POOL sequencer is otherwise idle) so
                # input and output streams don't serialize on one ring
                oeng = nc.gpsimd if ph % 2 == 0 else nc.sync
                for k in range(KC):
                    for j in range(JH):
                        nc.tensor.matmul(
                            pss[j][:],
                            m_sb[:, k, d0 : d0 + 128],
                            x_sb[(k, h)][:, j * 512 : (j + 1) * 512],
                            start=(k == 0),
                            stop=(k == KC - 1),
                        )
                if last:
                    # final phase: copy each 512-col bank on ACT, store each
                    # half on its own (idle, warm) HWDGE queue in parallel so
                    # the tail is one copy + one 128 KB store
                    nc.scalar.copy(ot[:, 0:512], pss[0][:])
                    nc.scalar.dma_start(
                        out=yt[d0 : d0 + 128, h * HW : h * HW + 512],
                        in_=ot[:, 0:512],
                    )
                    nc.scalar.copy(ot[:, 512:1024], pss[1][:])
                    nc.sync.dma_start(
                        out=yt[d0 : d0 + 128, h * HW + 512 : (h + 1) * HW],
                        in_=ot[:, 512:1024],
                    )
                else:
                    for j in range(JH):
                        nc.scalar.copy(ot[:, j * 512 : (j + 1) * 512], pss[j][:])
                    oeng.dma_start(
                        out=yt[d0 : d0 + 128, h * HW : (h + 1) * HW], in_=ot[:]
                    )
    nc.compile()
    return nc


def _fold_m(W_v, s_p, W_p, beta_p, W_o):
    """Host-side constant folding of the tiny parameter tensors into M."""
    W_v = np.asarray(W_v, dtype=np.float64)
    s_p = np.asarray(s_p, dtype=np.float64)
    W_p = np.asarray(W_p, dtype=np.float64)
    beta_p = np.asarray(beta_p, dtype=np.float64)
    W_o = np.asarray(W_o, dtype=np.float64)
    phi = np.einsum("h,dhc,d->hc", s_p, W_p, beta_p)
    norma = np.linalg.norm(phi, axis=1)  # [h]
    M = np.einsum("dhc,h,hce->de", W_v, norma, W_o)  # [512, 512]
    return M.astype(np.float32)


_prog_cache = {}
_last_in_maps = None  # kept for test.py profiling reuse
_last_result = None


def _run(in_maps, token, **kwargs):
    if token not in _prog_cache:
        _prog_cache[token] = _build(token)
    return run_bass_kernel_spmd(_prog_cache[token], in_maps, list(range(N_CORES)), **kwargs)


def kernel(x, W_v, s_p, c_p, W_p, W_A, W_o, beta_p, beta_i_p, **_unused):
    global _last_in_maps, _last_result
    token = COMPUTE_DTYPE
    np_dt = _np_dtype(token)

    x = np.asarray(x, dtype=np.float32)
    M = _fold_m(W_v, s_p, W_p, beta_p, W_o)

    # fp16 path: scale M by an exact power of two so M entries and y values
    # sit in fp16 normal range; undo on the host after the run
    out_unscale = 1.0
    if token == "fp16":
        amax = float(np.abs(M).max())
        if amax > 0:
            e = int(np.floor(-np.log2(amax)))
            M = M * np.float32(2.0**e)
            out_unscale = 2.0**-e

    B, N, Dd = x.shape
    assert B * N == ROWS and Dd == D, (x.shape,)

    mmc = np.ascontiguousarray(M.reshape(KC, 128, D).transpose(1, 0, 2)).astype(np_dt)
    xf = x.reshape(ROWS, D)

    in_maps = []
    for c in range(N_CORES):
        sh = xf[c * RPC : (c + 1) * RPC]               # [4096, 512]
        xT = sh.T.astype(np_dt)                        # [512, 4096]
        # [KC, 128, HB, HW] -> [KC, HB, 128, HW], each quarter contiguous
        xs = np.ascontiguousarray(
            xT.reshape(KC, 128, HB, HW).transpose(0, 2, 1, 3)
        )
        in_maps.append({"xt": xs, "mm": mmc})

    _last_in_maps = in_maps
    res = _run(in_maps, token)
    _last_result = res
    out = np.empty((ROWS, D), dtype=np.float32)
    for c in range(N_CORES):
        yc = res.results[c]["yt"].astype(np.float32)
        if out_unscale != 1.0:
            yc *= np.float32(out_unscale)
        out[c * RPC : (c + 1) * RPC] = yc.T
    return out.reshape(B, N, D)


if __name__ == "__main__":
    # smoke test with random data
    rng = np.random.default_rng(0)
    x = rng.standard_normal((8, 4096, 512)).astype(np.float32)
    W_v = rng.standard_normal((512, 8, 64)).astype(np.float32) * 0.01
    s_p = np.ones((8,), np.float32)
    c_p = np.ones((8,), np.float32)
    W_p = rng.standard_normal((512, 8, 64)).astype(np.float32) * 0.01
    W_A = rng.standard_normal((256, 64)).astype(np.float32)
    W_o = rng.standard_normal((8, 64, 512)).astype(np.float32) * 0.01
    beta_p = rng.standard_normal((512,)).astype(np.float32) * 1e-5
    beta_i_p = rng.standard_normal((4096, 512)).astype(np.float32) * 1e-5
    out = kernel(x, W_v=W_v, s_p=s_p, c_p=c_p, W_p=W_p, W_A=W_A, W_o=W_o,
                 beta_p=beta_p, beta_i_p=beta_i_p)
    M = _fold_m(W_v, s_p, W_p, beta_p, W_o)
    exp = (x.reshape(-1, 512).astype(np.float64) @ M.astype(np.float64)).reshape(8, 4096, 512)
    err = np.abs(out - exp).max() / (np.abs(exp).max() + 1e-30)
    print("smoke rel err:", err)
